# revision 10
# baseline (speedup 1.0000x reference)
"""Trainium2 Bass kernel for nn_A_Sp_P4CNN (p4-equivariant CNN with spatial
attention), data-parallel over 8 NeuronCores (batch 32 -> 4 per core).

Device layout: feature maps are SBUF tiles [128 partitions = g*32 + c,
free = (b_local, H, W)], bf16. Main 3x3 VALID convs = 9 accumulating
matmuls with windowed rhs APs, 4 output rotations packed via col-group
tile_position. Training-mode BatchNorm uses one 64-float AllReduce per
layer. 7x7 'same' attention convs contract an im2col tensor A112
(kx-parity x ky x 8ch on partitions) with 4 accumulating matmuls.
"""
import sys

sys.path.insert(0, '/opt/trn_rl_repo')
import numpy as np
import ml_dtypes
import concourse.bass as bass
import concourse.bacc as bacc
import concourse.tile as tile
import concourse.mybir as mybir
from concourse import bass_utils
from concourse.alu_op_type import AluOpType

F32 = mybir.dt.float32
BF16 = mybir.dt.bfloat16
AF = mybir.ActivationFunctionType
BF = ml_dtypes.bfloat16

N_CORES = 8
B, NG = 32, 4
BL = B // N_CORES
H0 = 48
EPS = 2e-5
GG_DIMS = [(46, 44), (22, 20), (20, 18), (18, 16), (16, 14), (14, 12)]
GG_CO = [32, 32, 32, 32, 32, 10]


def _rot(w, r):
    return np.rot90(w, k=r, axes=(-2, -1))


def _gg_rot(w, r):
    return np.roll(_rot(w, r), shift=r, axis=2)


def prep_weights(inp):
    out = {}
    k1a = (inp['aw1'][0, 0] + inp['aw1'][0, 1]).astype(np.float32)
    lhsT_a1 = np.zeros((49, 4), np.float32)
    for r in range(4):
        lhsT_a1[:, r] = _rot(k1a, r).reshape(49)
    out['lhsT_a1'] = lhsT_a1.astype(BF)

    w1 = np.asarray(inp['w1'], np.float32)
    lhsT_1 = np.zeros((128, 128), np.float32)
    for r in range(4):
        wr = _rot(w1, r)
        for tap in range(9):
            dy, dx = tap // 3, tap % 3
            lhsT_1[32 * r + tap, 32 * r:32 * r + 32] = wr[:, 0, dy, dx]
    out['lhsT_1'] = lhsT_1.astype(BF)

    for li in range(6):
        w = np.asarray(inp[f'w{li + 2}'], np.float32)
        aw = np.asarray(inp[f'aw{li + 2}'], np.float32)
        co = w.shape[0]
        lhsT = np.zeros((128, 4, 9, co), np.float32)
        for r in range(4):
            wr = _gg_rot(w, r)
            for tap in range(9):
                dy, dx = tap // 3, tap % 3
                lhsT[:, r, tap, :] = (
                    wr[:, :, :, dy, dx].transpose(2, 1, 0).reshape(128, co))
        out[f'lhsT_w{li + 2}'] = lhsT.astype(BF)
        lhsTa = np.zeros((112, 4, 4), np.float32)  # [k, j, r]
        for r in range(4):
            ar = _gg_rot(aw, r)[0]  # [2,4,7,7]
            for j in range(4):
                for kxp in range(2):
                    kx = 2 * j + kxp
                    if kx > 6:
                        continue
                    for ky in range(7):
                        for ch in range(2):
                            for g in range(4):
                                lhsTa[kxp * 56 + ky * 8 + ch * 4 + g, j, r] = \
                                    ar[ch, g, ky, kx]
        out[f'lhsT_a{li + 2}'] = lhsTa.astype(BF)
    mean_l = np.zeros((128, 4), np.float32)
    for g in range(4):
        mean_l[g * 32:(g + 1) * 32, g] = 1.0 / 32.0
    out['lhsT_mean'] = mean_l.astype(BF)
    for i in range(1, 7):
        out[f'gam{i}'] = np.asarray(inp[f'g{i}'], np.float32).reshape(32, 1)
        out[f'bet{i}'] = np.asarray(inp[f'be{i}'], np.float32).reshape(32, 1)
    out['b7'] = np.asarray(inp['b7'], np.float32).reshape(10, 1)
    return out


def rpc_of(w):
    return max(1, 512 // w)


def build_nc():
    nc = bacc.Bacc("TRN2", target_bir_lowering=False, debug=False,
                   num_devices=N_CORES)
    dts = {}

    def din(name, shape, dt=F32):
        dts[name] = nc.dram_tensor(name, list(shape), dt, kind="ExternalInput")

    din('xbf', (BL, 1, H0, H0), BF16)
    din('lhsT_a1', (49, 4), BF16)
    din('lhsT_1', (128, 128), BF16)
    for li in range(6):
        din(f'lhsT_w{li + 2}', (128, 4, 9, GG_CO[li]), BF16)
        din(f'lhsT_a{li + 2}', (112, 4, 4), BF16)
    din('lhsT_mean', (128, 4), BF16)
    for i in range(1, 7):
        din(f'gam{i}', (32, 1))
        din(f'bet{i}', (32, 1))
    din('b7', (10, 1))
    out_t = nc.dram_tensor("out", [BL, 10], F32, kind="ExternalOutput")

    NZ1 = BL * 46 * 46

    with tile.TileContext(nc) as tc:
        with tc.tile_pool(name="const", bufs=1) as cpool, \
             tc.tile_pool(name="maps", bufs=1) as mpool, \
             tc.tile_pool(name="work", bufs=1) as wpool, \
             tc.tile_pool(name="small", bufs=4) as spool, \
             tc.tile_pool(name="psA", bufs=4, space="PSUM") as psA, \
             tc.tile_pool(name="psB", bufs=2, space="PSUM") as psB, \
             tc.tile_pool(name="psC", bufs=2, space="PSUM") as psC, \
             tc.tile_pool(name="dram", bufs=1, space="DRAM") as dpool:

            # ---- constants ----
            w_l1a = cpool.tile([49, 4], BF16)
            nc.sync.dma_start(w_l1a[:], dts['lhsT_a1'].ap())
            w_l1 = cpool.tile([128, 128], BF16)
            nc.sync.dma_start(w_l1[:], dts['lhsT_1'].ap())
            w_gg, w_att = [], []
            for li in range(6):
                wt = cpool.tile([128, 4, 9, GG_CO[li]], BF16, tag=f"wgg{li}")
                nc.sync.dma_start(wt[:], dts[f'lhsT_w{li + 2}'].ap())
                w_gg.append(wt)
                at = cpool.tile([112, 4, 4], BF16, tag=f"watt{li}")
                nc.sync.dma_start(at[:], dts[f'lhsT_a{li + 2}'].ap())
                w_att.append(at)
            w_mean = cpool.tile([128, 4], BF16)
            nc.sync.dma_start(w_mean[:], dts['lhsT_mean'].ap())
            gams, bets = [], []
            for i in range(1, 7):
                g_ = cpool.tile([32, 1], F32, tag=f"gam{i}")
                nc.sync.dma_start(g_[:], dts[f'gam{i}'].ap())
                gams.append(g_)
                b_ = cpool.tile([32, 1], F32, tag=f"bet{i}")
                nc.sync.dma_start(b_[:], dts[f'bet{i}'].ap())
                bets.append(b_)
            b7s = cpool.tile([10, 1], F32)
            nc.sync.dma_start(b7s[:], dts['b7'].ap())

            # =========== c1 ===========
            xbfd = dts['xbf'].ap()
            A49 = mpool.tile([49, BL, H0, H0], BF16, tag="bigA")
            nc.gpsimd.memset(A49[:], 0.0)
            for ky in range(7):
                for kx in range(7):
                    ylo, yhi = max(0, 3 - ky), min(H0, H0 + 3 - ky)
                    xlo, xhi = max(0, 3 - kx), min(H0, H0 + 3 - kx)
                    for b in range(BL):
                        nc.sync.dma_start(
                            A49[ky * 7 + kx:ky * 7 + kx + 1, b,
                                ylo:yhi, xlo:xhi],
                            xbfd[b, 0, ylo + ky - 3:yhi + ky - 3,
                                 xlo + kx - 3:xhi + kx - 3].unsqueeze(0))
            att1 = mpool.tile([4, BL, H0, H0], BF16, tag="att1")
            rpc1 = rpc_of(H0)
            for b in range(BL):
                for y0 in range(0, H0, rpc1):
                    ny = min(rpc1, H0 - y0)
                    ps = psB.tile([4, 512], F32, tag="psatt")
                    pv = ps[:, 0:ny * H0]
                    nc.tensor.matmul(pv, w_l1a[:], A49[:, b, y0:y0 + ny, :],
                                     start=True, stop=True)
                    nc.scalar.activation(
                        att1[:, b, y0:y0 + ny, :].rearrange(
                            "p a b -> p (a b)"), pv, AF.Sigmoid)
            H1 = 46
            A36 = mpool.tile([128, BL, H1, H1], BF16, tag="tA")
            att36 = mpool.tile([128, BL, H1, H1], BF16, tag="bigA")
            nc.gpsimd.memset(A36[:], 0.0)
            nc.gpsimd.memset(att36[:], 0.0)
            for r in range(4):
                for tap in range(9):
                    dy, dx = tap // 3, tap % 3
                    p = 32 * r + tap
                    for b in range(BL):
                        nc.sync.dma_start(
                            A36[p:p + 1, b],
                            xbfd[b, 0, dy:dy + H1, dx:dx + H1].unsqueeze(0))
                        nc.sync.dma_start(
                            att36[p:p + 1, b],
                            att1[r:r + 1, b, dy:dy + H1, dx:dx + H1])
            yconv = mpool.tile([128, NZ1], BF16, tag="yconv")
            A36m = yconv[:].rearrange("p (a b c) -> p a b c", a=BL, b=H1)
            nc.vector.tensor_tensor(A36m, A36[:], att36[:], AluOpType.mult)

            z_cur = mpool.tile([128, BL, 46, 46], BF16, tag="zA")
            stats = spool.tile([128, 24, 6], F32, tag="stats")
            rpc = rpc_of(46)
            nch = 0
            for b in range(BL):
                for y0 in range(0, 46, rpc):
                    ny = min(rpc, 46 - y0)
                    ps = psA.tile([128, 512], F32, tag="psmain")
                    pv = ps[:, 0:ny * 46]
                    nc.tensor.matmul(pv, w_l1[:],
                                     A36m[:, b, y0:y0 + ny, :],
                                     start=True, stop=True)
                    off = b * 46 * 46 + y0 * 46
                    nc.scalar.activation(yconv[:, off:off + ny * 46], pv,
                                         AF.Copy)
                    nc.vector.bn_stats(stats[:, nch],
                                       yconv[:, off:off + ny * 46])
                    nch += 1

            def bn_apply(stats_t, nchunks, li, nloc, z_out, y_ap, nout_els):
                ag = spool.tile([128, 2], F32, tag="ag")
                nc.vector.bn_aggr(ag[:], stats_t[:, 0:nchunks])
                m2 = spool.tile([128, 2], F32, tag="m2")
                nc.vector.tensor_tensor(m2[:, 0:1], ag[:, 0:1], ag[:, 0:1],
                                        AluOpType.mult)
                nc.vector.tensor_tensor(m2[:, 1:2], ag[:, 1:2], m2[:, 0:1],
                                        AluOpType.add)
                sq = spool.tile([128, 2], F32, tag="sq")
                nc.vector.tensor_scalar_mul(sq[:, 0:1], ag[:, 0:1],
                                            float(nloc))
                nc.vector.tensor_scalar_mul(sq[:, 1:2], m2[:, 1:2],
                                            float(nloc))
                fs = spool.tile([64, 2], F32, tag="fs")
                nc.sync.dma_start(fs[:], sq[64:128, :])
                f1 = spool.tile([64, 2], F32, tag="f1")
                nc.vector.tensor_tensor(f1[:], sq[0:64, :], fs[:],
                                        AluOpType.add)
                fs2 = spool.tile([32, 2], F32, tag="fs2")
                nc.sync.dma_start(fs2[:], f1[32:64, :])
                f2 = spool.tile([32, 2], F32, tag="f2")
                nc.vector.tensor_tensor(f2[:], f1[0:32, :], fs2[:],
                                        AluOpType.add)
                cin = dpool.tile([32, 2], F32, tag=f"arin{li}")
                cout = dpool.tile([32, 2], F32, tag=f"arout{li}")
                nc.sync.dma_start(cin[:], f2[:])
                nc.gpsimd.collective_compute(
                    "AllReduce", AluOpType.add,
                    replica_groups=[list(range(N_CORES))],
                    ins=[cin.opt()], outs=[cout.opt()])
                gst = spool.tile([32, 2], F32, tag="gst")
                nc.sync.dma_start(gst[:], cout[:])
                ngl = float(nloc * N_CORES * 4)
                mn = spool.tile([32, 2], F32, tag="mn")
                nc.vector.tensor_scalar_mul(mn[:], gst[:], 1.0 / ngl)
                var = spool.tile([32, 1], F32, tag="var")
                nc.vector.tensor_tensor(var[:], mn[:, 0:1], mn[:, 0:1],
                                        AluOpType.mult)
                nc.vector.tensor_tensor(var[:], mn[:, 1:2], var[:],
                                        AluOpType.subtract)
                epst = spool.tile([32, 1], F32, tag="epst")
                nc.vector.memset(epst[:], EPS)
                sig = spool.tile([32, 1], F32, tag="sig")
                nc.scalar.activation(sig[:], var[:], AF.Sqrt, bias=epst[:])
                inv = spool.tile([32, 1], F32, tag="inv")
                nc.vector.reciprocal(inv[:], sig[:])
                sb = spool.tile([32, 2], F32, tag="sb")
                nc.vector.tensor_tensor(sb[:, 0:1], gams[li][:], inv[:],
                                        AluOpType.mult)
                nc.vector.tensor_tensor(sb[:, 1:2], mn[:, 0:1], sb[:, 0:1],
                                        AluOpType.mult)
                nc.vector.tensor_tensor(sb[:, 1:2], bets[li][:], sb[:, 1:2],
                                        AluOpType.subtract)
                sb128 = spool.tile([128, 2], F32, tag="sb128")
                for g in range(4):
                    nc.sync.dma_start(sb128[32 * g:32 * g + 32, :], sb[:])
                CH = 4096
                for o in range(0, nout_els, CH):
                    n = min(CH, nout_els - o)
                    nc.scalar.activation(z_out[:, o:o + n], y_ap[:, o:o + n],
                                         AF.Relu, bias=sb128[:, 1:2],
                                         scale=sb128[:, 0:1])

            bn_apply(stats, nch, 0, BL * 46 * 46,
                     z_cur[:].rearrange("p a b c -> p (a b c)"),
                     yconv[:], NZ1)

            shT = mpool.tile([128, NZ1], BF16, tag="shT")
            narrowA = mpool.tile([128, NZ1], BF16, tag="narrowA")
            bscr = narrowA[0:1]
            att4 = narrowA[32:36]
            amean = narrowA[64:68]
            tm4 = narrowA[96:100]
            nc.gpsimd.memset(shT[:], 0.0)
            nc.gpsimd.memset(narrowA[:], 0.0)

            # =========== GG layers ===========
            for li in range(6):
                hin, hout = GG_DIMS[li]
                co = GG_CO[li]
                win, wout = hin, hout
                nin = BL * hin * win
                nout = BL * hout * wout
                wpad = win + 6
                zf = z_cur[:].rearrange("p a b c -> p (a b c)")
                t_a = mpool.tile([128, NZ1], BF16, tag="tA")
                t_b = yconv
                if li == 0:
                    nc.gpsimd.memset(t_a[:], 0.0)

                # channel-max tree
                cur = zf
                for lev, shf in enumerate([16, 8, 4, 2, 1]):
                    for g in range(4):
                        nc.sync.dma_start(
                            shT[32 * g:32 * g + shf, 0:nin],
                            cur[32 * g + shf:32 * g + 2 * shf, 0:nin])
                    nxt = (t_a if lev % 2 == 0 else t_b)[:, 0:nin]
                    nc.vector.tensor_tensor(nxt, cur[:, 0:nin],
                                            shT[:, 0:nin], AluOpType.max)
                    cur = nxt
                for g in range(4):
                    nc.sync.dma_start(tm4[g:g + 1, 0:nin],
                                      cur[32 * g:32 * g + 1, 0:nin])

                # channel mean via PE
                rpci = rpc_of(win)
                for b in range(BL):
                    for y0 in range(0, hin, rpci):
                        ny = min(rpci, hin - y0)
                        ps = psC.tile([4, 512], F32, tag="psmean")
                        pv = ps[:, 0:ny * win]
                        nc.tensor.matmul(
                            pv, w_mean[:],
                            zf[:, (b * hin + y0) * win:
                               (b * hin + y0 + ny) * win],
                            start=True, stop=True)
                        off = (b * hin + y0) * win
                        nc.scalar.activation(amean[:, off:off + ny * win],
                                             pv, AF.Copy)

                # A112
                A112 = mpool.tile([112, BL, hin, wpad], BF16, tag="bigA")
                nc.gpsimd.memset(A112[:], 0.0)
                amv = amean[:, 0:nin].rearrange("p (a b c) -> p a b c",
                                                a=BL, b=hin)
                tmv = tm4[:, 0:nin].rearrange("p (a b c) -> p a b c",
                                              a=BL, b=hin)
                for kxp in range(2):
                    for ky in range(7):
                        pb = kxp * 56 + ky * 8
                        ylo, yhi = max(0, 3 - ky), min(hin, hin + 3 - ky)
                        xlo = 3 - kxp
                        xhi = min(wpad, win + 3 - kxp)
                        for b in range(BL):
                            nc.sync.dma_start(
                                A112[pb:pb + 4, b, ylo:yhi, xlo:xhi],
                                amv[:, b, ylo + ky - 3:yhi + ky - 3,
                                    0:xhi - xlo])
                            nc.sync.dma_start(
                                A112[pb + 4:pb + 8, b, ylo:yhi, xlo:xhi],
                                tmv[:, b, ylo + ky - 3:yhi + ky - 3,
                                    0:xhi - xlo])

                for b in range(BL):
                    for y0 in range(0, hin, rpci):
                        ny = min(rpci, hin - y0)
                        ps = psB.tile([4, 512], F32, tag="psatt")
                        pv = ps[:, 0:ny * win]
                        for j in range(4):
                            nc.tensor.matmul(
                                pv, w_att[li][:, j],
                                A112[:, b, y0:y0 + ny, 2 * j:2 * j + win],
                                start=(j == 0), stop=(j == 3))
                        nc.scalar.activation(
                            att4[:, (b * hin + y0) * win:
                                 (b * hin + y0 + ny) * win],
                            pv, AF.Sigmoid)

                # per-sample: broadcast, modulate, conv
                statsL = spool.tile([128, 24, 6], F32, tag="stats")
                nchL = 0
                rpco = rpc_of(wout)
                zv = z_cur
                for b in range(BL):
                    a128 = wpool.tile([128, 4, hin, win], BF16, tag="a128")
                    xr = wpool.tile([128, 4, hin, win], BF16, tag="xr")
                    for r in range(4):
                        nc.sync.dma_start(
                            bscr[:, 0:hin * win],
                            att4[r:r + 1,
                                 b * hin * win:(b + 1) * hin * win])
                        nc.gpsimd.partition_broadcast(
                            a128[:, r].rearrange("p a b -> p (a b)"),
                            bscr[:, 0:hin * win])
                        nc.vector.tensor_tensor(xr[:, r], zv[:, b],
                                                a128[:, r], AluOpType.mult)
                    for y0 in range(0, hout, rpco):
                        ny = min(rpco, hout - y0)
                        ps = psA.tile([128, 512], F32, tag="psmain")
                        for r in range(4):
                            for tap in range(9):
                                dy, dx = tap // 3, tap % 3
                                nc.tensor.matmul(
                                    ps[32 * r:32 * r + co, 0:ny * wout],
                                    w_gg[li][:, r, tap],
                                    xr[:, r, y0 + dy:y0 + dy + ny,
                                       dx:dx + wout],
                                    start=(tap == 0), stop=(tap == 8),
                                    tile_position=(0, 32 * r))
                        off = (b * hout + y0) * wout
                        nc.scalar.activation(yconv[:, off:off + ny * wout],
                                             ps[:, 0:ny * wout], AF.Copy)
                        if li < 5:
                            nc.vector.bn_stats(
                                statsL[:, nchL],
                                yconv[:, off:off + ny * wout])
                            nchL += 1

                if li < 5:
                    ztag = "zB" if li % 2 == 0 else "zA"
                    z_nxt = mpool.tile([128, BL, hout, wout], BF16, tag=ztag)
                    bn_apply(statsL, nchL, li + 1, BL * hout * wout,
                             z_nxt[:].rearrange("p a b c -> p (a b c)"),
                             yconv[:], nout)
                    if li == 0:
                        zp = mpool.tile([128, BL, 22, 22], BF16, tag="zA")
                        tp_ = mpool.tile([128, BL, 44, 22], BF16, tag="tA")
                        nc.vector.tensor_tensor(
                            tp_[:], z_nxt[:, :, :, 0::2],
                            z_nxt[:, :, :, 1::2], AluOpType.max)
                        nc.vector.tensor_tensor(
                            zp[:], tp_[:, :, 0::2, :], tp_[:, :, 1::2, :],
                            AluOpType.max)
                        z_cur = zp
                    else:
                        z_cur = z_nxt
                else:
                    hs = spool.tile([64, BL * 144], BF16, tag="hs")
                    nc.sync.dma_start(hs[:], yconv[64:128, 0:nout])
                    h1 = spool.tile([64, BL * 144], BF16, tag="h1")
                    nc.vector.tensor_tensor(h1[:], yconv[0:64, 0:nout],
                                            hs[:], AluOpType.max)
                    hs2 = spool.tile([32, BL * 144], BF16, tag="hs2")
                    nc.sync.dma_start(hs2[:], h1[32:64])
                    h2 = spool.tile([32, BL * 144], BF16, tag="h2")
                    nc.vector.tensor_tensor(h2[:], h1[0:32], hs2[:],
                                            AluOpType.max)
                    red = spool.tile([10, BL], F32, tag="red")
                    nc.vector.tensor_reduce(
                        red[:],
                        h2[0:10].rearrange("p (a s) -> p a s", a=BL),
                        mybir.AxisListType.X, AluOpType.add)
                    fin = spool.tile([10, BL], F32, tag="fin")
                    nc.vector.tensor_scalar_mul(fin[:], red[:], 1.0 / 144.0)
                    fin2 = spool.tile([10, BL], F32, tag="fin2")
                    nc.vector.tensor_scalar(fin2[:], fin[:], b7s[:], None,
                                            AluOpType.add)
                    nc.sync.dma_start(out_t.ap().rearrange("b c -> c b"),
                                      fin2[:])

    nc.compile()
    return nc


_CACHE = {}


def get_nc():
    if 'nc' not in _CACHE:
        _CACHE['nc'] = build_nc()
    return _CACHE['nc']


def make_in_maps(inputs):
    prep = prep_weights(inputs)
    x = np.asarray(inputs['x'], np.float32).astype(BF)
    in_maps = []
    for c in range(N_CORES):
        m = dict(prep)
        m['xbf'] = np.ascontiguousarray(x[c * BL:(c + 1) * BL])
        in_maps.append(m)
    return in_maps


def kernel(**inputs):
    nc = get_nc()
    in_maps = make_in_maps(inputs)
    res = bass_utils.run_bass_kernel_spmd(
        nc, in_maps, core_ids=list(range(N_CORES)))
    return np.concatenate(
        [res.results[c]['out'] for c in range(N_CORES)], 0).astype(np.float32)


# revision 12
# speedup vs baseline: 1.2643x; 1.2643x over previous
"""Trainium2 Bass kernel for nn_A_Sp_P4CNN (p4-equivariant CNN with spatial
attention), data-parallel over 8 NeuronCores (batch 32 -> 4 per core).

Device layout: feature maps are SBUF tiles [128 partitions = g*32 + c,
free = (b_local, H, W)], bf16. Main 3x3 VALID convs = 9 accumulating
matmuls with windowed rhs APs, 4 output rotations packed via col-group
tile_position. Training-mode BatchNorm uses one 64-float AllReduce per
layer. 7x7 'same' attention convs contract an im2col tensor A112
(kx-parity x ky x 8ch on partitions) with 4 accumulating matmuls.
"""
import sys

sys.path.insert(0, '/opt/trn_rl_repo')
import numpy as np
import ml_dtypes
import concourse.bass as bass
import concourse.bacc as bacc
import concourse.tile as tile
import concourse.mybir as mybir
from concourse import bass_utils
from concourse.alu_op_type import AluOpType

F32 = mybir.dt.float32
BF16 = mybir.dt.bfloat16
AF = mybir.ActivationFunctionType
BF = ml_dtypes.bfloat16

N_CORES = 8
B, NG = 32, 4
BL = B // N_CORES
H0 = 48
EPS = 2e-5
GG_DIMS = [(46, 44), (22, 20), (20, 18), (18, 16), (16, 14), (14, 12)]
GG_CO = [32, 32, 32, 32, 32, 10]


def _rot(w, r):
    return np.rot90(w, k=r, axes=(-2, -1))


def _gg_rot(w, r):
    return np.roll(_rot(w, r), shift=r, axis=2)


def prep_weights(inp):
    out = {}
    k1a = (inp['aw1'][0, 0] + inp['aw1'][0, 1]).astype(np.float32)
    lhsT_a1 = np.zeros((49, 4), np.float32)
    for r in range(4):
        lhsT_a1[:, r] = _rot(k1a, r).reshape(49)
    out['lhsT_a1'] = lhsT_a1.astype(BF)

    w1 = np.asarray(inp['w1'], np.float32)
    lhsT_1 = np.zeros((128, 128), np.float32)
    for r in range(4):
        wr = _rot(w1, r)
        for tap in range(9):
            dy, dx = tap // 3, tap % 3
            lhsT_1[32 * r + tap, 32 * r:32 * r + 32] = wr[:, 0, dy, dx]
    out['lhsT_1'] = lhsT_1.astype(BF)

    for li in range(6):
        w = np.asarray(inp[f'w{li + 2}'], np.float32)
        aw = np.asarray(inp[f'aw{li + 2}'], np.float32)
        co = w.shape[0]
        lhsT = np.zeros((128, 4, 9, co), np.float32)
        for r in range(4):
            wr = _gg_rot(w, r)
            for tap in range(9):
                dy, dx = tap // 3, tap % 3
                lhsT[:, r, tap, :] = (
                    wr[:, :, :, dy, dx].transpose(2, 1, 0).reshape(128, co))
        out[f'lhsT_w{li + 2}'] = lhsT.astype(BF)
        lhsTa = np.zeros((112, 4, 4), np.float32)  # [k, j, r]
        for r in range(4):
            ar = _gg_rot(aw, r)[0]  # [2,4,7,7]
            for j in range(4):
                for kxp in range(2):
                    kx = 2 * j + kxp
                    if kx > 6:
                        continue
                    for ky in range(7):
                        for ch in range(2):
                            for g in range(4):
                                lhsTa[kxp * 56 + ky * 8 + ch * 4 + g, j, r] = \
                                    ar[ch, g, ky, kx]
        out[f'lhsT_a{li + 2}'] = lhsTa.astype(BF)
    mean_l = np.zeros((128, 4), np.float32)
    for g in range(4):
        mean_l[g * 32:(g + 1) * 32, g] = 1.0 / 32.0
    out['lhsT_mean'] = mean_l.astype(BF)
    for i in range(1, 7):
        out[f'gam{i}'] = np.asarray(inp[f'g{i}'], np.float32).reshape(32, 1)
        out[f'bet{i}'] = np.asarray(inp[f'be{i}'], np.float32).reshape(32, 1)
    out['b7'] = np.asarray(inp['b7'], np.float32).reshape(10, 1)
    return out


def rpc_of(w):
    return max(1, 512 // w)


def build_nc():
    nc = bacc.Bacc("TRN2", target_bir_lowering=False, debug=False,
                   num_devices=N_CORES)
    dts = {}

    def din(name, shape, dt=F32):
        dts[name] = nc.dram_tensor(name, list(shape), dt, kind="ExternalInput")

    din('xbf', (BL, 1, H0, H0), BF16)
    din('lhsT_a1', (49, 4), BF16)
    din('lhsT_1', (128, 128), BF16)
    for li in range(6):
        din(f'lhsT_w{li + 2}', (128, 4, 9, GG_CO[li]), BF16)
        din(f'lhsT_a{li + 2}', (112, 4, 4), BF16)
    din('lhsT_mean', (128, 4), BF16)
    for i in range(1, 7):
        din(f'gam{i}', (32, 1))
        din(f'bet{i}', (32, 1))
    din('b7', (10, 1))
    out_t = nc.dram_tensor("out", [BL, 10], F32, kind="ExternalOutput")

    NZ1 = BL * 46 * 46

    _dma_rr = [0]

    def dma(dst, src_):
        eng = (nc.sync, nc.scalar, nc.gpsimd)[_dma_rr[0] % 3]
        _dma_rr[0] += 1
        eng.dma_start(dst, src_)

    with tile.TileContext(nc) as tc:
        with tc.tile_pool(name="const", bufs=1) as cpool, \
             tc.tile_pool(name="maps", bufs=1) as mpool, \
             tc.tile_pool(name="work", bufs=1) as wpool, \
             tc.tile_pool(name="small", bufs=4) as spool, \
             tc.tile_pool(name="psA", bufs=4, space="PSUM") as psA, \
             tc.tile_pool(name="psB", bufs=2, space="PSUM") as psB, \
             tc.tile_pool(name="psC", bufs=2, space="PSUM") as psC, \
             tc.tile_pool(name="dram", bufs=1, space="DRAM") as dpool:

            # ---- constants ----
            w_l1a = cpool.tile([49, 4], BF16)
            dma(w_l1a[:], dts['lhsT_a1'].ap())
            w_l1 = cpool.tile([128, 128], BF16)
            dma(w_l1[:], dts['lhsT_1'].ap())
            w_gg, w_att = [], []
            for li in range(6):
                wt = cpool.tile([128, 4, 9, GG_CO[li]], BF16, tag=f"wgg{li}")
                dma(wt[:], dts[f'lhsT_w{li + 2}'].ap())
                w_gg.append(wt)
                at = cpool.tile([112, 4, 4], BF16, tag=f"watt{li}")
                dma(at[:], dts[f'lhsT_a{li + 2}'].ap())
                w_att.append(at)
            w_mean = cpool.tile([128, 4], BF16)
            dma(w_mean[:], dts['lhsT_mean'].ap())
            gams, bets = [], []
            for i in range(1, 7):
                g_ = cpool.tile([32, 1], F32, tag=f"gam{i}")
                dma(g_[:], dts[f'gam{i}'].ap())
                gams.append(g_)
                b_ = cpool.tile([32, 1], F32, tag=f"bet{i}")
                dma(b_[:], dts[f'bet{i}'].ap())
                bets.append(b_)
            b7s = cpool.tile([10, 1], F32)
            dma(b7s[:], dts['b7'].ap())

            # =========== c1 ===========
            xbfd = dts['xbf'].ap()
            A49 = mpool.tile([49, BL, H0, H0], BF16, tag="bigA")
            nc.gpsimd.memset(A49[:], 0.0)
            for ky in range(7):
                for kx in range(7):
                    ylo, yhi = max(0, 3 - ky), min(H0, H0 + 3 - ky)
                    xlo, xhi = max(0, 3 - kx), min(H0, H0 + 3 - kx)
                    for b in range(BL):
                        dma(
                            A49[ky * 7 + kx:ky * 7 + kx + 1, b,
                                ylo:yhi, xlo:xhi],
                            xbfd[b, 0, ylo + ky - 3:yhi + ky - 3,
                                 xlo + kx - 3:xhi + kx - 3].unsqueeze(0))
            att1 = mpool.tile([4, BL, H0, H0], BF16, tag="att1")
            rpc1 = rpc_of(H0)
            for b in range(BL):
                for y0 in range(0, H0, rpc1):
                    ny = min(rpc1, H0 - y0)
                    ps = psB.tile([4, 512], F32, tag="psatt")
                    pv = ps[:, 0:ny * H0]
                    nc.tensor.matmul(pv, w_l1a[:], A49[:, b, y0:y0 + ny, :],
                                     start=True, stop=True)
                    nc.scalar.activation(
                        att1[:, b, y0:y0 + ny, :].rearrange(
                            "p a b -> p (a b)"), pv, AF.Sigmoid)
            H1 = 46
            A36 = mpool.tile([128, BL, H1, H1], BF16, tag="tA")
            att36 = mpool.tile([128, BL, H1, H1], BF16, tag="bigA")
            nc.gpsimd.memset(A36[:], 0.0)
            nc.gpsimd.memset(att36[:], 0.0)
            for r in range(4):
                for tap in range(9):
                    dy, dx = tap // 3, tap % 3
                    p = 32 * r + tap
                    for b in range(BL):
                        dma(
                            A36[p:p + 1, b],
                            xbfd[b, 0, dy:dy + H1, dx:dx + H1].unsqueeze(0))
                        dma(
                            att36[p:p + 1, b],
                            att1[r:r + 1, b, dy:dy + H1, dx:dx + H1])
            yconv = mpool.tile([128, NZ1], BF16, tag="yconv")
            A36m = yconv[:].rearrange("p (a b c) -> p a b c", a=BL, b=H1)
            nc.vector.tensor_tensor(A36m, A36[:], att36[:], AluOpType.mult)

            z_cur = mpool.tile([128, BL, 46, 46], BF16, tag="zA")
            stats = spool.tile([128, 24, 6], F32, tag="stats")
            rpc = rpc_of(46)
            nch = 0
            for b in range(BL):
                for y0 in range(0, 46, rpc):
                    ny = min(rpc, 46 - y0)
                    ps = psA.tile([128, 512], F32, tag="psmain")
                    pv = ps[:, 0:ny * 46]
                    nc.tensor.matmul(pv, w_l1[:],
                                     A36m[:, b, y0:y0 + ny, :],
                                     start=True, stop=True)
                    off = b * 46 * 46 + y0 * 46
                    nc.scalar.activation(yconv[:, off:off + ny * 46], pv,
                                         AF.Copy)
                    nc.vector.bn_stats(stats[:, nch],
                                       yconv[:, off:off + ny * 46])
                    nch += 1

            def bn_apply(stats_t, nchunks, li, nloc, z_out, y_ap, nout_els):
                ag = spool.tile([128, 2], F32, tag="ag")
                nc.vector.bn_aggr(ag[:], stats_t[:, 0:nchunks])
                m2 = spool.tile([128, 2], F32, tag="m2")
                nc.vector.tensor_tensor(m2[:, 0:1], ag[:, 0:1], ag[:, 0:1],
                                        AluOpType.mult)
                nc.vector.tensor_tensor(m2[:, 1:2], ag[:, 1:2], m2[:, 0:1],
                                        AluOpType.add)
                sq = spool.tile([128, 2], F32, tag="sq")
                nc.vector.tensor_scalar_mul(sq[:, 0:1], ag[:, 0:1],
                                            float(nloc))
                nc.vector.tensor_scalar_mul(sq[:, 1:2], m2[:, 1:2],
                                            float(nloc))
                fs = spool.tile([64, 2], F32, tag="fs")
                dma(fs[:], sq[64:128, :])
                f1 = spool.tile([64, 2], F32, tag="f1")
                nc.vector.tensor_tensor(f1[:], sq[0:64, :], fs[:],
                                        AluOpType.add)
                fs2 = spool.tile([32, 2], F32, tag="fs2")
                dma(fs2[:], f1[32:64, :])
                f2 = spool.tile([32, 2], F32, tag="f2")
                nc.vector.tensor_tensor(f2[:], f1[0:32, :], fs2[:],
                                        AluOpType.add)
                cin = dpool.tile([32, 2], F32, tag=f"arin{li}")
                cout = dpool.tile([32, 2], F32, tag=f"arout{li}")
                dma(cin[:], f2[:])
                nc.gpsimd.collective_compute(
                    "AllReduce", AluOpType.add,
                    replica_groups=[list(range(N_CORES))],
                    ins=[cin.opt()], outs=[cout.opt()])
                gst = spool.tile([32, 2], F32, tag="gst")
                dma(gst[:], cout[:])
                ngl = float(nloc * N_CORES * 4)
                mn = spool.tile([32, 2], F32, tag="mn")
                nc.vector.tensor_scalar_mul(mn[:], gst[:], 1.0 / ngl)
                var = spool.tile([32, 1], F32, tag="var")
                nc.vector.tensor_tensor(var[:], mn[:, 0:1], mn[:, 0:1],
                                        AluOpType.mult)
                nc.vector.tensor_tensor(var[:], mn[:, 1:2], var[:],
                                        AluOpType.subtract)
                epst = spool.tile([32, 1], F32, tag="epst")
                nc.vector.memset(epst[:], EPS)
                sig = spool.tile([32, 1], F32, tag="sig")
                nc.scalar.activation(sig[:], var[:], AF.Sqrt, bias=epst[:])
                inv = spool.tile([32, 1], F32, tag="inv")
                nc.vector.reciprocal(inv[:], sig[:])
                sb = spool.tile([32, 2], F32, tag="sb")
                nc.vector.tensor_tensor(sb[:, 0:1], gams[li][:], inv[:],
                                        AluOpType.mult)
                nc.vector.tensor_tensor(sb[:, 1:2], mn[:, 0:1], sb[:, 0:1],
                                        AluOpType.mult)
                nc.vector.tensor_tensor(sb[:, 1:2], bets[li][:], sb[:, 1:2],
                                        AluOpType.subtract)
                sb128 = spool.tile([128, 2], F32, tag="sb128")
                for g in range(4):
                    dma(sb128[32 * g:32 * g + 32, :], sb[:])
                CH = 4096
                for o in range(0, nout_els, CH):
                    n = min(CH, nout_els - o)
                    nc.scalar.activation(z_out[:, o:o + n], y_ap[:, o:o + n],
                                         AF.Relu, bias=sb128[:, 1:2],
                                         scale=sb128[:, 0:1])

            bn_apply(stats, nch, 0, BL * 46 * 46,
                     z_cur[:].rearrange("p a b c -> p (a b c)"),
                     yconv[:], NZ1)

            shT = mpool.tile([128, NZ1], BF16, tag="shT")
            narrowA = mpool.tile([128, NZ1], BF16, tag="narrowA")
            bscr = narrowA[0:1]
            att4 = narrowA[32:36]
            amean = narrowA[64:68]
            tm4 = narrowA[96:100]
            nc.gpsimd.memset(shT[:], 0.0)
            nc.gpsimd.memset(narrowA[:], 0.0)

            # =========== GG layers ===========
            for li in range(6):
                hin, hout = GG_DIMS[li]
                co = GG_CO[li]
                win, wout = hin, hout
                nin = BL * hin * win
                nout = BL * hout * wout
                wpad = win + 6
                zf = z_cur[:].rearrange("p a b c -> p (a b c)")
                t_a = mpool.tile([128, NZ1], BF16, tag="tA")
                t_b = yconv
                if li == 0:
                    nc.gpsimd.memset(t_a[:], 0.0)

                # channel-max tree
                cur = zf
                for lev, shf in enumerate([16, 8, 4, 2, 1]):
                    for g in range(4):
                        dma(
                            shT[32 * g:32 * g + shf, 0:nin],
                            cur[32 * g + shf:32 * g + 2 * shf, 0:nin])
                    nxt = (t_a if lev % 2 == 0 else t_b)[:, 0:nin]
                    nc.vector.tensor_tensor(nxt, cur[:, 0:nin],
                                            shT[:, 0:nin], AluOpType.max)
                    cur = nxt
                for g in range(4):
                    dma(tm4[g:g + 1, 0:nin],
                                      cur[32 * g:32 * g + 1, 0:nin])

                # channel mean via PE
                rpci = rpc_of(win)
                for b in range(BL):
                    for y0 in range(0, hin, rpci):
                        ny = min(rpci, hin - y0)
                        ps = psC.tile([4, 512], F32, tag="psmean")
                        pv = ps[:, 0:ny * win]
                        nc.tensor.matmul(
                            pv, w_mean[:],
                            zf[:, (b * hin + y0) * win:
                               (b * hin + y0 + ny) * win],
                            start=True, stop=True)
                        off = (b * hin + y0) * win
                        nc.scalar.activation(amean[:, off:off + ny * win],
                                             pv, AF.Copy)

                # A112
                A112 = mpool.tile([112, BL, hin, wpad], BF16, tag="bigA")
                nc.gpsimd.memset(A112[:], 0.0)
                amv = amean[:, 0:nin].rearrange("p (a b c) -> p a b c",
                                                a=BL, b=hin)
                tmv = tm4[:, 0:nin].rearrange("p (a b c) -> p a b c",
                                              a=BL, b=hin)
                for kxp in range(2):
                    for ky in range(7):
                        pb = kxp * 56 + ky * 8
                        ylo, yhi = max(0, 3 - ky), min(hin, hin + 3 - ky)
                        xlo = 3 - kxp
                        xhi = min(wpad, win + 3 - kxp)
                        for b in range(BL):
                            dma(
                                A112[pb:pb + 4, b, ylo:yhi, xlo:xhi],
                                amv[:, b, ylo + ky - 3:yhi + ky - 3,
                                    0:xhi - xlo])
                            dma(
                                A112[pb + 4:pb + 8, b, ylo:yhi, xlo:xhi],
                                tmv[:, b, ylo + ky - 3:yhi + ky - 3,
                                    0:xhi - xlo])

                for b in range(BL):
                    for y0 in range(0, hin, rpci):
                        ny = min(rpci, hin - y0)
                        ps = psB.tile([4, 512], F32, tag="psatt")
                        pv = ps[:, 0:ny * win]
                        for j in range(4):
                            nc.tensor.matmul(
                                pv, w_att[li][:, j],
                                A112[:, b, y0:y0 + ny, 2 * j:2 * j + win],
                                start=(j == 0), stop=(j == 3))
                        nc.scalar.activation(
                            att4[:, (b * hin + y0) * win:
                                 (b * hin + y0 + ny) * win],
                            pv, AF.Sigmoid)

                # per-sample: broadcast, modulate, conv
                statsL = spool.tile([128, 24, 6], F32, tag="stats")
                nchL = 0
                rpco = rpc_of(wout)
                zv = z_cur
                for b in range(BL):
                    a128 = wpool.tile([128, 4, hin, win], BF16, tag="a128")
                    xr = wpool.tile([128, 4, hin, win], BF16, tag="xr")
                    for r in range(4):
                        dma(
                            bscr[:, 0:hin * win],
                            att4[r:r + 1,
                                 b * hin * win:(b + 1) * hin * win])
                        nc.gpsimd.partition_broadcast(
                            a128[:, r].rearrange("p a b -> p (a b)"),
                            bscr[:, 0:hin * win])
                        nc.vector.tensor_tensor(xr[:, r], zv[:, b],
                                                a128[:, r], AluOpType.mult)
                    for y0 in range(0, hout, rpco):
                        ny = min(rpco, hout - y0)
                        ps = psA.tile([128, 512], F32, tag="psmain")
                        for r in range(4):
                            for tap in range(9):
                                dy, dx = tap // 3, tap % 3
                                nc.tensor.matmul(
                                    ps[32 * r:32 * r + co, 0:ny * wout],
                                    w_gg[li][:, r, tap],
                                    xr[:, r, y0 + dy:y0 + dy + ny,
                                       dx:dx + wout],
                                    start=(tap == 0), stop=(tap == 8),
                                    tile_position=(0, 32 * r))
                        off = (b * hout + y0) * wout
                        nc.scalar.activation(yconv[:, off:off + ny * wout],
                                             ps[:, 0:ny * wout], AF.Copy)
                        if li < 5:
                            nc.vector.bn_stats(
                                statsL[:, nchL],
                                yconv[:, off:off + ny * wout])
                            nchL += 1

                if li < 5:
                    ztag = "zB" if li % 2 == 0 else "zA"
                    z_nxt = mpool.tile([128, BL, hout, wout], BF16, tag=ztag)
                    bn_apply(statsL, nchL, li + 1, BL * hout * wout,
                             z_nxt[:].rearrange("p a b c -> p (a b c)"),
                             yconv[:], nout)
                    if li == 0:
                        zp = mpool.tile([128, BL, 22, 22], BF16, tag="zA")
                        tp_ = mpool.tile([128, BL, 44, 22], BF16, tag="tA")
                        nc.vector.tensor_tensor(
                            tp_[:], z_nxt[:, :, :, 0::2],
                            z_nxt[:, :, :, 1::2], AluOpType.max)
                        nc.vector.tensor_tensor(
                            zp[:], tp_[:, :, 0::2, :], tp_[:, :, 1::2, :],
                            AluOpType.max)
                        z_cur = zp
                    else:
                        z_cur = z_nxt
                else:
                    hs = spool.tile([64, BL * 144], BF16, tag="hs")
                    dma(hs[:], yconv[64:128, 0:nout])
                    h1 = spool.tile([64, BL * 144], BF16, tag="h1")
                    nc.vector.tensor_tensor(h1[:], yconv[0:64, 0:nout],
                                            hs[:], AluOpType.max)
                    hs2 = spool.tile([32, BL * 144], BF16, tag="hs2")
                    dma(hs2[:], h1[32:64])
                    h2 = spool.tile([32, BL * 144], BF16, tag="h2")
                    nc.vector.tensor_tensor(h2[:], h1[0:32], hs2[:],
                                            AluOpType.max)
                    red = spool.tile([10, BL], F32, tag="red")
                    nc.vector.tensor_reduce(
                        red[:],
                        h2[0:10].rearrange("p (a s) -> p a s", a=BL),
                        mybir.AxisListType.X, AluOpType.add)
                    fin = spool.tile([10, BL], F32, tag="fin")
                    nc.vector.tensor_scalar_mul(fin[:], red[:], 1.0 / 144.0)
                    fin2 = spool.tile([10, BL], F32, tag="fin2")
                    nc.vector.tensor_scalar(fin2[:], fin[:], b7s[:], None,
                                            AluOpType.add)
                    dma(out_t.ap().rearrange("b c -> c b"),
                                      fin2[:])

    nc.compile()
    return nc


_CACHE = {}


def get_nc():
    if 'nc' not in _CACHE:
        _CACHE['nc'] = build_nc()
    return _CACHE['nc']


def make_in_maps(inputs):
    prep = prep_weights(inputs)
    x = np.asarray(inputs['x'], np.float32).astype(BF)
    in_maps = []
    for c in range(N_CORES):
        m = dict(prep)
        m['xbf'] = np.ascontiguousarray(x[c * BL:(c + 1) * BL])
        in_maps.append(m)
    return in_maps


def kernel(**inputs):
    nc = get_nc()
    in_maps = make_in_maps(inputs)
    res = bass_utils.run_bass_kernel_spmd(
        nc, in_maps, core_ids=list(range(N_CORES)))
    return np.concatenate(
        [res.results[c]['out'] for c in range(N_CORES)], 0).astype(np.float32)
